# revision 4
# baseline (speedup 1.0000x reference)
"""AttentionProtoNet pooling kernel for 8x TRN2 NeuronCores.

reference (per sample of B=64, L=512, H=768):
    upsilon = tanh(hs @ W_fc.T + b_fc)        [L, H]
    nu      = upsilon @ W_nu                  [L]
    alphas  = softmax(nu)                     [L]
    pooled  = alphas @ hs                     [H]

Strategy: data-parallel over B (8 samples per core). The big GEMM runs in
bf16 (1 cycle/row on the PE, same rate as f32r but half the HBM traffic
and SBUF) against a single bf16 X^T copy that also feeds the pooling
stage. Output channels are pre-sorted by |W_nu| on the host (upsilon only
exists to produce nu = W_nu . tanh(...), so channel precision is weighted
by |W_nu|) which lets later variants drop low-|W_nu| channels to fp8.
tanh runs on ACT straight out of PSUM (per-partition bias), nu is a bf16
matmul against the tanh output, softmax on 1 partition with bf16 exp,
alphas broadcast across partitions via GpSimd, weighted-sum pooling on
the VectorEngine in bf16 (2x mode), outputs drain per-sample through a
tiny PE transpose. Sample s's nu/softmax/pool work is emitted after
sample s+1's GEMM so the PE never waits on the ACT pipeline.
"""

import sys

sys.path.insert(0, "/opt/trn_rl_repo")

import numpy as np
import ml_dtypes

B, L, H = 64, 512, 768
NCORES = 8
SPC = B // NCORES            # samples per core
HC = H // 128                # 128-partition chunks of H
WARMUP_MM = 10               # junk matmuls to ramp PE pstate during DMA

_compiled = {}


def _build():
    import concourse.bass as bass
    import concourse.bacc as bacc
    import concourse.tile as tile
    from concourse import mybir
    from concourse.masks import make_identity

    F32 = mybir.dt.float32
    BF16 = mybir.dt.bfloat16
    AF = mybir.ActivationFunctionType
    ALU = mybir.AluOpType

    nc = bacc.Bacc(None, target_bir_lowering=False)

    # host layouts (see kernel()):
    #  xb [128, SPC, HC, L] bf16  : bf16(X^T[128j+p, 512s+l])
    #  wb [128, HC, HC, 128] bf16 : wb[p,hc,t,m] = WT[128hc+p, ord[128t+m]]
    #  bfc [128, HC] f32          : b_fc[ord[128k+p]]
    #  wnu [128, HC] bf16         : W_nu[ord[128k+p]]
    xb_d = nc.dram_tensor("xb", [128, SPC, HC, L], BF16, kind="ExternalInput")
    wb_d = nc.dram_tensor("wb", [128, HC, HC, 128], BF16, kind="ExternalInput")
    bfc_d = nc.dram_tensor("bfc", [128, HC], F32, kind="ExternalInput")
    wnu_d = nc.dram_tensor("wnu", [128, HC], BF16, kind="ExternalInput")
    out_d = nc.dram_tensor("out", [SPC, H], F32, kind="ExternalOutput")
    junk_d = nc.dram_tensor("junk", [128, 8], F32)   # warmup sink

    with tile.TileContext(nc) as tc:
        with tc.tile_pool(name="xp", bufs=1) as xp, \
             tc.tile_pool(name="wp", bufs=1) as wp, \
             tc.tile_pool(name="cst", bufs=1) as cst, \
             tc.tile_pool(name="ups", bufs=2) as upsp, \
             tc.tile_pool(name="sm", bufs=2) as smp, \
             tc.tile_pool(name="outp", bufs=2) as outp, \
             tc.tile_pool(name="mmps", bufs=5, space="PSUM") as mmps, \
             tc.tile_pool(name="nups", bufs=2, space="PSUM") as nups, \
             tc.tile_pool(name="tps", bufs=1, space="PSUM") as tps:

            # ---- PE warmup: junk matmuls with no DMA dependency ramp the
            # PE pstate while the first tiles stream in.
            wu_sb = cst.tile([128, 512], BF16)
            nc.vector.memset(wu_sb[:], 1.0)
            wu_ps = tps.tile([128, 512], F32, tag="tp", name="wu_ps")
            for i in range(WARMUP_MM):
                nc.tensor.matmul(wu_ps[:], wu_sb[:, 0:128], wu_sb[:],
                                 start=(i == 0), stop=(i == WARMUP_MM - 1))
            wu_out = cst.tile([128, 8], F32)
            nc.scalar.copy(wu_out[:], wu_ps[:, 0:8])
            nc.sync.dma_start(junk_d[:], wu_out[:])

            # ---- constants + weights (sync queue, before X)
            bfc_sb = cst.tile([128, HC], F32)
            nc.sync.dma_start(bfc_sb[:], bfc_d[:])
            wnu_sb = cst.tile([128, HC], BF16)
            nc.sync.dma_start(wnu_sb[:], wnu_d[:])
            wb_sb = wp.tile([128, HC, HC, 128], BF16, name="wb")
            nc.sync.dma_start(wb_sb[:, :, 0], wb_d[:, :, 0])  # t=0 first
            ident = cst.tile([128, 128], F32)
            make_identity(nc, ident[:])
            nc.sync.dma_start(wb_sb[:, :, 1:HC], wb_d[:, :, 1:HC])

            xb_sb = xp.tile([128, SPC, HC, L], BF16, name="xb")
            # X spread over three DMA queues; sample 0 on gpsimd so the
            # sync queue's weight loads don't delay it.
            qmap = {0: nc.gpsimd, 1: nc.scalar, 2: nc.sync, 3: nc.gpsimd,
                    4: nc.scalar, 5: nc.sync, 6: nc.gpsimd, 7: nc.scalar}
            for s in range(SPC):
                qmap[s].dma_start(xb_sb[:, s], xb_d[:, s])

            # ---- per-sample pipeline; tail (nu onward) of sample s is
            # emitted after sample s+1's GEMM to keep the PE dense.
            def gemm(s):
                ups = upsp.tile([128, HC, L], BF16, tag="ups")
                for t in range(HC):
                    ps = mmps.tile([128, L], F32, tag="mm")
                    for hc in range(HC):
                        nc.tensor.matmul(
                            ps[:], wb_sb[:, hc, t], xb_sb[:, s, hc, :],
                            start=(hc == 0), stop=(hc == HC - 1),
                        )
                    nc.scalar.activation(
                        ups[:, t, :], ps[:], AF.Tanh,
                        bias=bfc_sb[:, t:t + 1],
                    )
                return ups

            def tail(s, ups):
                nu = nups.tile([1, L], F32, tag="nu")
                for k in range(HC):
                    nc.tensor.matmul(
                        nu[:], wnu_sb[:, k:k + 1], ups[:, k, :],
                        start=(k == 0), stop=(k == HC - 1),
                    )

                # softmax over the 512 logits (single partition); nu is
                # small enough that exp() needs no max subtraction
                ex = smp.tile([1, L], BF16, tag="ex")
                z = smp.tile([1, 1], F32, tag="z")
                nc.scalar.activation(ex[:], nu[:], AF.Exp, accum_out=z[:])
                rz = smp.tile([1, 1], F32, tag="rz")
                nc.vector.reciprocal(rz[:], z[:])

                # broadcast unnormalized E (recip runs in parallel), pool
                # in bf16 (2x DVE), then scale pooled by 1/Z
                ab = smp.tile([128, L], BF16, tag="ab")
                nc.gpsimd.partition_broadcast(ab[:], ex[:])
                rzb = smp.tile([HC, 1], F32, tag="rzb")
                nc.gpsimd.partition_broadcast(rzb[:], rz[:], channels=HC)
                pooled_u = outp.tile([128, HC], F32, tag="pooled_u")
                for j in range(HC):
                    trash = smp.tile([128, L], BF16, tag="trash")
                    nc.vector.scalar_tensor_tensor(
                        trash[:],
                        xb_sb[:, s, j, :],
                        1.0,
                        ab[:],
                        ALU.mult,
                        ALU.mult,
                        accum_out=pooled_u[:, j:j + 1],
                    )

                # pooled^T [128, HC] -> [HC, 128] -> DRAM row s
                tp = tps.tile([HC, 128], F32, tag="tp")
                nc.tensor.transpose(tp[:], pooled_u[:], ident[:])
                orow = outp.tile([HC, 128], F32, tag="orow")
                nc.scalar.activation(orow[:], tp[:], AF.Copy,
                                     scale=rzb[:, 0:1])
                nc.sync.dma_start(
                    out_d[s:s + 1, :].rearrange("o (c p) -> (o c) p", p=128),
                    orow[:],
                )

            prev = None
            for s in range(SPC):
                ups = gemm(s)
                if prev is not None:
                    tail(s - 1, prev)
                prev = ups
            tail(SPC - 1, prev)

    nc.finalize()
    return nc


def _prep_host(hidden_states, W_fc, b_fc, W_nu):
    bf = ml_dtypes.bfloat16
    hs = np.ascontiguousarray(hidden_states, dtype=np.float32)
    W_fc = np.asarray(W_fc, np.float32)
    b_fc = np.asarray(b_fc, np.float32)
    W_nu = np.asarray(W_nu, np.float32)

    order = np.argsort(np.abs(W_nu), kind="stable")
    WT = W_fc.T[:, order]                                # [hin, kout sorted]

    # wb[p, hc, t, m] = WT[128hc+p, 128t+m]
    wb = WT.reshape(HC, 128, HC, 128)
    wb = np.ascontiguousarray(wb.transpose(1, 0, 2, 3)).astype(bf)
    bfc = np.ascontiguousarray(b_fc[order].reshape(HC, 128).T, np.float32)
    wnu = np.ascontiguousarray(W_nu[order].reshape(HC, 128).T.astype(bf))

    # per-core X^T in [p, s, j, l] layout
    xbs = []
    for c in range(NCORES):
        xt = hs[c * SPC:(c + 1) * SPC].reshape(SPC * L, H).T  # [H, TOK]
        v = xt.reshape(HC, 128, SPC, L).transpose(1, 2, 0, 3)  # [p,s,j,l]
        xbs.append(np.ascontiguousarray(v).astype(bf))
    return wb, bfc, wnu, xbs


def kernel(hidden_states, W_fc, b_fc, W_nu, _trace=False, _trace_kwargs=None):
    from concourse.bass_utils import run_bass_kernel_spmd

    wb, bfc, wnu, xbs = _prep_host(hidden_states, W_fc, b_fc, W_nu)
    in_maps = [
        {"xb": xbs[c], "wb": wb, "bfc": bfc, "wnu": wnu}
        for c in range(NCORES)
    ]

    if "nc" not in _compiled:
        _compiled["nc"] = _build()
    res = run_bass_kernel_spmd(
        _compiled["nc"], in_maps, list(range(NCORES)),
        trace=_trace, **(_trace_kwargs or {}),
    )
    kernel.last_results = res
    out = np.concatenate([np.asarray(r["out"], np.float32) for r in res.results])
    return out


# revision 5
# speedup vs baseline: 1.0245x; 1.0245x over previous
"""AttentionProtoNet pooling kernel for 8x TRN2 NeuronCores.

reference (per sample of B=64, L=512, H=768):
    upsilon = tanh(hs @ W_fc.T + b_fc)        [L, H]
    nu      = upsilon @ W_nu                  [L]
    alphas  = softmax(nu)                     [L]
    pooled  = alphas @ hs                     [H]

Strategy: data-parallel over B (8 samples per core). The big GEMM runs in
bf16 (1 cycle/row on the PE at the full 2.4 GHz pstate) against a single
bf16 X^T copy that also feeds the pooling stage. Weights are shipped
t-major so each 128-kout chunk is one contiguous-per-partition DMA, with
the biases and W_nu (as bf16) packed into the head of the first weight
transfer; sample 0's X streams as six per-chunk DMAs so the first matmul
chain starts as data lands. All on the sync queue, which boots first;
later samples ride the gpsimd/scalar queues. tanh runs on ACT straight
out of PSUM (per-partition bias), nu is a bf16 matmul against the tanh
output, softmax on 1 partition with bf16 exp, alphas broadcast across
partitions via GpSimd, weighted-sum pooling on the VectorEngine in bf16,
and outputs drain per-sample through a tiny PE transpose. Sample s's
tail (nu/softmax/pool) is interleaved into sample s+1's GEMM block so
the PE stream stays dense and the post-GEMM tail is one sample deep.
"""

import sys

sys.path.insert(0, "/opt/trn_rl_repo")

import numpy as np
import ml_dtypes

B, L, H = 64, 512, 768
NCORES = 8
SPC = B // NCORES            # samples per core
HC = H // 128                # 128-partition chunks of H
CW = 2 * HC                  # const columns at the head of wbig (bfc, wnu)
WARMUP_MM = 12               # junk matmuls to ramp PE pstate during DMA

_compiled = {}


def _build():
    import concourse.bass as bass
    import concourse.bacc as bacc
    import concourse.tile as tile
    from concourse import mybir
    from concourse.masks import make_identity

    F32 = mybir.dt.float32
    BF16 = mybir.dt.bfloat16
    AF = mybir.ActivationFunctionType
    ALU = mybir.AluOpType

    nc = bacc.Bacc(None, target_bir_lowering=False)

    # host layouts (see kernel()):
    #  xb [128, SPC, HC, L] bf16 : bf16(X^T[128j+p, 512s+l])
    #  wbig [128, CW + HC*768]   : cols 0:HC = b_fc[ord[128t+p]] (bf16),
    #    cols HC:2HC = W_nu[ord[128t+p]], then t-major weights:
    #    wbig[p, CW + t*768 + hc*128 + m] = WT[128hc+p, ord[128t+m]]
    xb_d = nc.dram_tensor("xb", [128, SPC, HC, L], BF16, kind="ExternalInput")
    wbig_d = nc.dram_tensor("wbig", [128, CW + HC * H], BF16,
                            kind="ExternalInput")
    out_d = nc.dram_tensor("out", [SPC, H], F32, kind="ExternalOutput")
    junk_d = nc.dram_tensor("junk", [128, 8], F32)   # warmup sink

    with tile.TileContext(nc) as tc:
        with tc.tile_pool(name="xp", bufs=1) as xp, \
             tc.tile_pool(name="wp", bufs=1) as wp, \
             tc.tile_pool(name="cst", bufs=1) as cst, \
             tc.tile_pool(name="ups", bufs=3) as upsp, \
             tc.tile_pool(name="sm", bufs=4) as smp, \
             tc.tile_pool(name="outp", bufs=4) as outp, \
             tc.tile_pool(name="mmps", bufs=6, space="PSUM") as mmps, \
             tc.tile_pool(name="nups", bufs=1, space="PSUM") as nups, \
             tc.tile_pool(name="tps", bufs=1, space="PSUM") as tps:

            # ---- PE warmup: junk matmuls with no DMA dependency ramp the
            # PE pstate while the first tiles stream in.
            wu_sb = cst.tile([128, 512], BF16)
            nc.vector.memset(wu_sb[:], 1.0)
            wu_ps = tps.tile([128, 512], F32, tag="tp", name="wu_ps")
            for i in range(WARMUP_MM):
                nc.tensor.matmul(wu_ps[:], wu_sb[:, 0:128], wu_sb[:],
                                 start=(i == 0), stop=(i == WARMUP_MM - 1))
            wu_out = cst.tile([128, 8], F32)
            nc.scalar.copy(wu_out[:], wu_ps[:, 0:8])

            wbig_sb = wp.tile([128, CW + HC * H], BF16, name="wbig")
            xb_sb = xp.tile([128, SPC, HC, L], BF16, name="xb")
            ident = cst.tile([128, 128], F32)
            make_identity(nc, ident[:])

            def wslice(t):
                a = CW + t * H
                return slice(a, a + H)

            # critical prologue, all on the sync queue (first to boot):
            # consts + t0 weights, sample-0 X per chunk racing the t0 GEMM
            # chain, remaining weight chunks just ahead of their use.
            nc.sync.dma_start(wbig_sb[:, 0:CW + H], wbig_d[:, 0:CW + H])
            for j in range(3):
                nc.sync.dma_start(xb_sb[:, 0, j], xb_d[:, 0, j])
            nc.sync.dma_start(wbig_sb[:, wslice(1)], wbig_d[:, wslice(1)])
            for j in range(3, HC):
                nc.sync.dma_start(xb_sb[:, 0, j], xb_d[:, 0, j])
            for t in range(2, HC):
                nc.sync.dma_start(wbig_sb[:, wslice(t)], wbig_d[:, wslice(t)])
            # bulk: remaining samples on the side queues (+ one on sync)
            nc.sync.dma_start(xb_sb[:, 2], xb_d[:, 2])
            for s in (1, 3, 5, 7):
                nc.gpsimd.dma_start(xb_sb[:, s], xb_d[:, s])
            for s in (4, 6):
                nc.scalar.dma_start(xb_sb[:, s], xb_d[:, s])

            def mm_tiles(s, ups, trange):
                for t in trange:
                    ps = mmps.tile([128, L], F32, tag="mm")
                    for hc in range(HC):
                        nc.tensor.matmul(
                            ps[:],
                            wbig_sb[:, CW + t * H + hc * 128:
                                    CW + t * H + hc * 128 + 128],
                            xb_sb[:, s, hc, :],
                            start=(hc == 0), stop=(hc == HC - 1),
                        )
                    nc.scalar.activation(
                        ups[:, t, :], ps[:], AF.Tanh,
                        bias=wbig_sb[:, t:t + 1],
                    )

            def tail_nu(s, ups):
                nu = nups.tile([1, L], F32, tag="nu")
                for k in range(HC):
                    nc.tensor.matmul(
                        nu[:], wbig_sb[:, HC + k:HC + k + 1], ups[:, k, :],
                        start=(k == 0), stop=(k == HC - 1),
                    )
                return nu

            def tail_rest(s, nu):
                # softmax over the 512 logits (single partition); nu is
                # small enough that exp() needs no max subtraction
                ex = smp.tile([1, L], BF16, tag="ex")
                z = smp.tile([1, 1], F32, tag="z")
                nc.scalar.activation(ex[:], nu[:], AF.Exp, accum_out=z[:])
                rz = smp.tile([1, 1], F32, tag="rz")
                nc.vector.reciprocal(rz[:], z[:])

                # broadcast unnormalized E, pool in bf16, scale by 1/Z
                ab = smp.tile([128, L], BF16, tag="ab")
                nc.gpsimd.partition_broadcast(ab[:], ex[:])
                rzb = smp.tile([HC, 1], F32, tag="rzb")
                nc.gpsimd.partition_broadcast(rzb[:], rz[:], channels=HC)
                pooled_u = outp.tile([128, HC], F32, tag="pooled_u")
                for j in range(HC):
                    trash = smp.tile([128, L], BF16, tag="trash")
                    nc.vector.scalar_tensor_tensor(
                        trash[:],
                        xb_sb[:, s, j, :],
                        1.0,
                        ab[:],
                        ALU.mult,
                        ALU.mult,
                        accum_out=pooled_u[:, j:j + 1],
                    )

                # pooled^T [128, HC] -> [HC, 128] -> DRAM row s
                tp = tps.tile([HC, 128], F32, tag="tp")
                nc.tensor.transpose(tp[:], pooled_u[:], ident[:])
                orow = outp.tile([HC, 128], F32, tag="orow")
                nc.scalar.activation(orow[:], tp[:], AF.Copy,
                                     scale=rzb[:, 0:1])
                nc.sync.dma_start(
                    out_d[s:s + 1, :].rearrange("o (c p) -> (o c) p", p=128),
                    orow[:],
                )

            prev = None   # (s, ups, nu) of the in-flight previous sample
            for s in range(SPC):
                ups = upsp.tile([128, HC, L], BF16, tag="ups")
                mm_tiles(s, ups, range(0, 2))
                if prev is not None:
                    pnu = tail_nu(prev[0], prev[1])
                mm_tiles(s, ups, range(2, HC))
                if prev is not None:
                    tail_rest(prev[0], pnu)
                prev = (s, ups)
            pnu = tail_nu(prev[0], prev[1])
            tail_rest(prev[0], pnu)

            nc.sync.dma_start(junk_d[:], wu_out[:])

    nc.finalize()
    return nc


def _prep_host(hidden_states, W_fc, b_fc, W_nu):
    bf = ml_dtypes.bfloat16
    hs = np.ascontiguousarray(hidden_states, dtype=np.float32)
    W_fc = np.asarray(W_fc, np.float32)
    b_fc = np.asarray(b_fc, np.float32)
    W_nu = np.asarray(W_nu, np.float32)

    order = np.argsort(np.abs(W_nu), kind="stable")
    WT = W_fc.T[:, order]                                # [hin, kout sorted]

    # wbig: [bfc | wnu | t-major W]
    wbig = np.empty((128, CW + HC * H), dtype=bf)
    wbig[:, 0:HC] = b_fc[order].reshape(HC, 128).T.astype(bf)
    wbig[:, HC:CW] = W_nu[order].reshape(HC, 128).T.astype(bf)
    # wbig[p, CW + t*768 + hc*128 + m] = WT[128hc+p, ord[128t+m]]
    w = WT.reshape(HC, 128, HC, 128).transpose(1, 2, 0, 3)  # [p, t, hc, m]
    wbig[:, CW:] = np.ascontiguousarray(w).reshape(128, HC * H).astype(bf)

    # per-core X^T in [p, s, j, l] layout
    xbs = []
    for c in range(NCORES):
        xt = hs[c * SPC:(c + 1) * SPC].reshape(SPC * L, H).T  # [H, TOK]
        v = xt.reshape(HC, 128, SPC, L).transpose(1, 2, 0, 3)  # [p,s,j,l]
        xbs.append(np.ascontiguousarray(v).astype(bf))
    return wbig, xbs


def kernel(hidden_states, W_fc, b_fc, W_nu, _trace=False, _trace_kwargs=None):
    from concourse.bass_utils import run_bass_kernel_spmd

    wbig, xbs = _prep_host(hidden_states, W_fc, b_fc, W_nu)
    in_maps = [{"xb": xbs[c], "wbig": wbig} for c in range(NCORES)]

    if "nc" not in _compiled:
        _compiled["nc"] = _build()
    res = run_bass_kernel_spmd(
        _compiled["nc"], in_maps, list(range(NCORES)),
        trace=_trace, **(_trace_kwargs or {}),
    )
    kernel.last_results = res
    out = np.concatenate([np.asarray(r["out"], np.float32) for r in res.results])
    return out


# revision 11
# speedup vs baseline: 1.1122x; 1.0857x over previous
"""AttentionProtoNet pooling kernel for 8x TRN2 NeuronCores.

reference (per sample of B=64, L=512, H=768):
    upsilon = tanh(hs @ W_fc.T + b_fc)        [L, H]
    nu      = upsilon @ W_nu                  [L]
    alphas  = softmax(nu)                     [L]
    pooled  = alphas @ hs                     [H]

Strategy: data-parallel over B (8 samples per core). The big GEMM runs in
bf16 (1 cycle/row on the PE at the full 2.4 GHz pstate) against a single
bf16 X^T copy that also feeds the pooling stage. Output channels are
sorted by |W_nu| on the host: upsilon only exists to produce the scalar
nu = W_nu . tanh(...), so the 512 lowest-|W_nu| channels can round their
tanh output to fp8e4 with negligible effect, letting the nu contraction
run as two fp8 DoubleRow matmuls (256-deep, 0.5 cyc/row) plus two bf16
ones - half the PE cost of a pure bf16 nu. W_nu rides along x64 (fp8
needs the scale to stay normal; exp() folds 1/64 back in for free).
Weights + biases + W_nu ship as ONE contiguous-per-partition DMA (small
strided lines run at ~30 GB/s vs ~170 GB/s for 8KB lines); X samples are
monolithic per-sample transfers split across the gpsimd/scalar queues.
tanh runs on ACT straight out of PSUM (per-partition bias), softmax on 1
partition with bf16 exp, alphas broadcast via GpSimd, weighted-sum
pooling on the VectorEngine in bf16, outputs drain per-sample through a
tiny PE transpose. Sample s's tail (nu/softmax/pool) is interleaved into
sample s+1's GEMM block so the PE stream stays dense.
"""

import sys

sys.path.insert(0, "/opt/trn_rl_repo")

import numpy as np
import ml_dtypes

B, L, H = 64, 512, 768
NCORES = 8
SPC = B // NCORES            # samples per core
HC = H // 128                # 128-partition chunks of H
N8C = int(__import__("os").environ.get("N8C", "4"))  # fp8 ups chunks
NUM = 8                      # wnu occupies m=0 of each 16B-strided plane
CW = HC + HC + N8C * 16 // 2   # head cols: bias | wnu*64 bf16 | packed fp8
WARMUP_MM = 18               # junk matmuls bridge PE to first data

_compiled = {}


def _build():
    import concourse.bass as bass
    import concourse.bacc as bacc
    import concourse.tile as tile
    from concourse import mybir
    from concourse.masks import make_identity

    F32 = mybir.dt.float32
    BF16 = mybir.dt.bfloat16
    F8 = mybir.dt.float8e4
    AF = mybir.ActivationFunctionType
    ALU = mybir.AluOpType
    DR = mybir.MatmulPerfMode.DoubleRow

    nc = bacc.Bacc(None, target_bir_lowering=False)

    # host layouts (see kernel()):
    #  xb [128, SPC, HC, L] bf16 : bf16(X^T[128j+p, 512s+l])
    #  wbig [128, CW + HC*768] bf16:
    #    cols 0:6   = b_fc[ord[128t+p]]
    #    cols 6:12  = 64*W_nu[ord[128t+p]]
    #    cols 12:44 = fp8 bytes [u, i, 16]: byte 0 of each 16B plane
    #                 holds fp8(64*W_nu[ord[128*(2u+i)+p]]), rest zero
    #                 (dual-fp8 ldweights needs >=8B segments and >=16B
    #                 plane stride)
    #    then t-major weights:
    #    wbig[p, CW + t*768 + hc*128 + m] = WT[128hc+p, ord[128t+m]]
    xb_d = nc.dram_tensor("xb", [128, SPC, HC, L], BF16, kind="ExternalInput")
    wbig_d = nc.dram_tensor("wbig", [128, CW + HC * H], BF16,
                            kind="ExternalInput")
    out_d = nc.dram_tensor("out", [SPC, H], F32, kind="ExternalOutput")
    junk_d = nc.dram_tensor("junk", [128, 8], F32)   # warmup sink

    with tile.TileContext(nc) as tc:
        with tc.tile_pool(name="xp", bufs=1) as xp, \
             tc.tile_pool(name="wp", bufs=1) as wp, \
             tc.tile_pool(name="cst", bufs=1) as cst, \
             tc.tile_pool(name="ups", bufs=3) as upsp, \
             tc.tile_pool(name="sm", bufs=4) as smp, \
             tc.tile_pool(name="outp", bufs=4) as outp, \
             tc.tile_pool(name="mmps", bufs=6, space="PSUM") as mmps, \
             tc.tile_pool(name="nups", bufs=1, space="PSUM") as nups, \
             tc.tile_pool(name="tps", bufs=1, space="PSUM") as tps:

            # ---- PE warmup: junk matmuls with no DMA dependency keep the
            # PE pstate ramping while wbig + sample 0 stream in.
            wu_sb = cst.tile([128, 512], BF16)
            nc.vector.memset(wu_sb[:], 1.0)
            wu_ps = tps.tile([128, 512], F32, tag="tp", name="wu_ps")
            for i in range(WARMUP_MM):
                nc.tensor.matmul(wu_ps[:], wu_sb[:, 0:128], wu_sb[:],
                                 start=(i == 0), stop=(i == WARMUP_MM - 1))
            wu_out = cst.tile([128, 8], F32)
            nc.scalar.copy(wu_out[:], wu_ps[:, 0:8])

            wbig_sb = wp.tile([128, CW + HC * H], BF16, name="wbig")
            xb_sb = xp.tile([128, SPC, HC, L], BF16, name="xb")
            ident = cst.tile([128, 128], F32)
            make_identity(nc, ident[:])

            # one big-line transfer for all weights/consts on the sync
            # queue; X per-sample monolithic on the side queues
            nc.sync.dma_start(wbig_sb[:], wbig_d[:])
            for s in range(4):
                nc.gpsimd.dma_start(xb_sb[:, s], xb_d[:, s])
            for s in range(4, SPC):
                nc.scalar.dma_start(xb_sb[:, s], xb_d[:, s])

            # [128, u, i, m] fp8 view of the packed W_nu head: planes
            # are 16B apart, ldweights reads the first 8 of each
            wnu8 = None
            if N8C:
                wnu8 = wbig_sb[:, 2 * HC:CW].bitcast(F8).rearrange(
                    "p (u i m) -> p u i m", u=N8C // 2, i=2)[:, :, :, 0:NUM]

            def mm_tiles(s, ups8, upsb, trange):
                for t in trange:
                    ps = mmps.tile([128, L], F32, tag="mm")
                    for hc in range(HC):
                        nc.tensor.matmul(
                            ps[:],
                            wbig_sb[:, CW + t * H + hc * 128:
                                    CW + t * H + hc * 128 + 128],
                            xb_sb[:, s, hc, :],
                            start=(hc == 0), stop=(hc == HC - 1),
                        )
                    dst = ups8[:, t, :] if t < N8C else upsb[:, t - N8C, :]
                    nc.scalar.activation(
                        dst, ps[:], AF.Tanh, bias=wbig_sb[:, t:t + 1],
                    )

            def tail_nu(s, ups8, upsb):
                # nu*64: two fp8 DoubleRow passes over the low-|W_nu|
                # chunks + two bf16 passes over the high ones
                nu = nups.tile([NUM, L], F32, tag="nu")
                if N8C == 0:
                    nc.tensor.matmul(
                        nu[0:1, :], wbig_sb[:, HC:HC + 1], upsb[:, 0, :],
                        start=True, stop=False, skip_group_check=True,
                    )
                for u in range(N8C // 2):
                    nc.tensor.matmul(
                        nu[:], wnu8[:, u],
                        ups8[:, 2 * u:2 * u + 2, :],
                        start=(u == 0), stop=False,
                        perf_mode=DR, skip_group_check=True,
                    )
                for k in range(max(N8C, 1), HC):
                    nc.tensor.matmul(
                        nu[0:1, :], wbig_sb[:, HC + k:HC + k + 1],
                        upsb[:, k - N8C, :],
                        start=False, stop=(k == HC - 1),
                        skip_group_check=True,
                    )
                return nu

            def tail_rest(s, nu):
                # softmax over the 512 logits (single partition); exp's
                # scale folds away the x64 on W_nu. logits are small
                # enough that exp() needs no max subtraction.
                ex = smp.tile([1, L], BF16, tag="ex")
                z = smp.tile([1, 1], F32, tag="z")
                nc.scalar.activation(ex[:], nu[0:1, :], AF.Exp,
                                     scale=1.0 / 64.0, accum_out=z[:])
                rz = smp.tile([1, 1], F32, tag="rz")
                nc.vector.reciprocal(rz[:], z[:])

                # broadcast unnormalized E, pool in bf16, scale by 1/Z
                ab = smp.tile([128, L], BF16, tag="ab")
                nc.gpsimd.partition_broadcast(ab[:], ex[:])
                rzb = smp.tile([HC, 1], F32, tag="rzb")
                nc.gpsimd.partition_broadcast(rzb[:], rz[:], channels=HC)
                pooled_u = outp.tile([128, HC], F32, tag="pooled_u")
                for j in range(HC):
                    trash = smp.tile([128, L], BF16, tag="trash")
                    nc.vector.scalar_tensor_tensor(
                        trash[:],
                        xb_sb[:, s, j, :],
                        1.0,
                        ab[:],
                        ALU.mult,
                        ALU.mult,
                        accum_out=pooled_u[:, j:j + 1],
                    )

                # pooled^T [128, HC] -> [HC, 128] -> DRAM row s
                tp = tps.tile([HC, 128], F32, tag="tp")
                nc.tensor.transpose(tp[:], pooled_u[:], ident[:])
                orow = outp.tile([HC, 128], F32, tag="orow")
                nc.scalar.activation(orow[:], tp[:], AF.Copy,
                                     scale=rzb[:, 0:1])
                nc.sync.dma_start(
                    out_d[s:s + 1, :].rearrange("o (c p) -> (o c) p", p=128),
                    orow[:],
                )

            prev = None
            for s in range(SPC):
                ups8 = None
                if N8C:
                    ups8 = upsp.tile([128, N8C, L], F8, tag="ups8", name="ups8")
                upsb = upsp.tile([128, HC - N8C, L], BF16, tag="upsb")
                mm_tiles(s, ups8, upsb, range(0, 2))
                if prev is not None:
                    pnu = tail_nu(prev[0], prev[1], prev[2])
                mm_tiles(s, ups8, upsb, range(2, HC))
                if prev is not None:
                    tail_rest(prev[0], pnu)
                prev = (s, ups8, upsb)
            pnu = tail_nu(prev[0], prev[1], prev[2])
            tail_rest(prev[0], pnu)

            nc.sync.dma_start(junk_d[:], wu_out[:])

    nc.finalize()
    return nc


def _prep_host(hidden_states, W_fc, b_fc, W_nu):
    bf = ml_dtypes.bfloat16
    f8 = ml_dtypes.float8_e4m3fn
    hs = np.ascontiguousarray(hidden_states, dtype=np.float32)
    W_fc = np.asarray(W_fc, np.float32)
    b_fc = np.asarray(b_fc, np.float32)
    W_nu = np.asarray(W_nu, np.float32)

    order = np.argsort(np.abs(W_nu), kind="stable")
    WT = W_fc.T[:, order]                                # [hin, kout sorted]
    wnu64 = (W_nu[order] * 64.0).reshape(HC, 128)        # [t, p]

    wbig = np.empty((128, CW + HC * H), dtype=bf)
    wbig[:, 0:HC] = b_fc[order].reshape(HC, 128).T.astype(bf)
    wbig[:, HC:2 * HC] = wnu64.T.astype(bf)
    # packed fp8 wnu for the DoubleRow nu: 16B [u, i] planes, wnu at m=0
    pk = wnu64[0:N8C].astype(f8).view(np.uint8)          # [4 chunks, 128]
    head = np.zeros((128, N8C, 16), np.uint8)
    head[:, :, 0] = pk.T
    wbig[:, 2 * HC:CW].view(np.uint8)[:] = head.reshape(128, N8C * 16)
    w = WT.reshape(HC, 128, HC, 128).transpose(1, 2, 0, 3)  # [p, t, hc, m]
    wbig[:, CW:] = np.ascontiguousarray(w).reshape(128, HC * H).astype(bf)

    xbs = []
    for c in range(NCORES):
        xt = hs[c * SPC:(c + 1) * SPC].reshape(SPC * L, H).T  # [H, TOK]
        v = xt.reshape(HC, 128, SPC, L).transpose(1, 2, 0, 3)  # [p,s,j,l]
        xbs.append(np.ascontiguousarray(v).astype(bf))
    return wbig, xbs


def kernel(hidden_states, W_fc, b_fc, W_nu, _trace=False, _trace_kwargs=None):
    from concourse.bass_utils import run_bass_kernel_spmd

    wbig, xbs = _prep_host(hidden_states, W_fc, b_fc, W_nu)
    in_maps = [{"xb": xbs[c], "wbig": wbig} for c in range(NCORES)]

    if "nc" not in _compiled:
        _compiled["nc"] = _build()
    res = run_bass_kernel_spmd(
        _compiled["nc"], in_maps, list(range(NCORES)),
        trace=_trace, **(_trace_kwargs or {}),
    )
    kernel.last_results = res
    out = np.concatenate([np.asarray(r["out"], np.float32) for r in res.results])
    return out


# revision 12
# speedup vs baseline: 1.1725x; 1.0541x over previous
"""AttentionProtoNet pooling kernel for 8x TRN2 NeuronCores.

reference (per sample of B=64, L=512, H=768):
    upsilon = tanh(hs @ W_fc.T + b_fc)        [L, H]
    nu      = upsilon @ W_nu                  [L]
    alphas  = softmax(nu)                     [L]
    pooled  = alphas @ hs                     [H]

Strategy: data-parallel over B (8 samples per core). The big GEMM runs in
bf16 (1 cycle/row on the PE at the full 2.4 GHz pstate) against a single
bf16 X^T copy that also feeds the pooling stage. Output channels are
sorted by |W_nu| on the host: upsilon only exists to produce the scalar
nu = W_nu . tanh(...), so the 512 lowest-|W_nu| channels can round their
tanh output to fp8e4 with negligible effect, letting the nu contraction
run as two fp8 DoubleRow matmuls (256-deep, 0.5 cyc/row) plus two bf16
ones - half the PE cost of a pure bf16 nu. W_nu rides along x64 (fp8
needs the scale to stay normal; exp() folds 1/64 back in for free).
Weights + biases + W_nu ship as ONE contiguous-per-partition DMA (small
strided lines run at ~30 GB/s vs ~170 GB/s for 8KB lines); X samples are
monolithic per-sample transfers split across the gpsimd/scalar queues.
tanh runs on ACT straight out of PSUM (per-partition bias), softmax on 1
partition with bf16 exp, alphas broadcast via GpSimd, weighted-sum
pooling on the VectorEngine in bf16, outputs drain per-sample through a
tiny PE transpose. Sample s's tail (nu/softmax/pool) is interleaved into
sample s+1's GEMM block so the PE stream stays dense.
"""

import sys

sys.path.insert(0, "/opt/trn_rl_repo")

import numpy as np
import ml_dtypes

B, L, H = 64, 512, 768
NCORES = 8
SPC = B // NCORES            # samples per core
HC = H // 128                # 128-partition chunks of H
N8C = int(__import__("os").environ.get("N8C", "4"))  # fp8 ups chunks
NUM = 8                      # wnu occupies m=0 of each 16B-strided plane
CW = HC + HC + N8C * 16 // 2   # head cols: bias | wnu*64 bf16 | packed fp8
WARMUP_MM = 22               # junk matmuls bridge PE to first data

_compiled = {}


def _build():
    import concourse.bass as bass
    import concourse.bacc as bacc
    import concourse.tile as tile
    from concourse import mybir
    from concourse.masks import make_identity

    F32 = mybir.dt.float32
    BF16 = mybir.dt.bfloat16
    F8 = mybir.dt.float8e4
    AF = mybir.ActivationFunctionType
    ALU = mybir.AluOpType
    DR = mybir.MatmulPerfMode.DoubleRow

    nc = bacc.Bacc(None, target_bir_lowering=False)

    # host layouts (see kernel()):
    #  xb [128, SPC, HC, L] bf16 : bf16(X^T[128j+p, 512s+l])
    #  wbig [128, CW + HC*768] bf16:
    #    cols 0:6   = b_fc[ord[128t+p]]
    #    cols 6:12  = 64*W_nu[ord[128t+p]]
    #    cols 12:44 = fp8 bytes [u, i, 16]: byte 0 of each 16B plane
    #                 holds fp8(64*W_nu[ord[128*(2u+i)+p]]), rest zero
    #                 (dual-fp8 ldweights needs >=8B segments and >=16B
    #                 plane stride)
    #    then t-major weights:
    #    wbig[p, CW + t*768 + hc*128 + m] = WT[128hc+p, ord[128t+m]]
    xb_d = nc.dram_tensor("xb", [128, SPC, HC, L], BF16, kind="ExternalInput")
    wbig_d = nc.dram_tensor("wbig", [128, CW + HC * H], BF16,
                            kind="ExternalInput")
    out_d = nc.dram_tensor("out", [SPC, H], F32, kind="ExternalOutput")
    junk_d = nc.dram_tensor("junk", [128, 8], F32)   # warmup sink

    with tile.TileContext(nc) as tc:
        with tc.tile_pool(name="xp", bufs=1) as xp, \
             tc.tile_pool(name="wp", bufs=1) as wp, \
             tc.tile_pool(name="cst", bufs=1) as cst, \
             tc.tile_pool(name="ups", bufs=3) as upsp, \
             tc.tile_pool(name="sm", bufs=4) as smp, \
             tc.tile_pool(name="outp", bufs=4) as outp, \
             tc.tile_pool(name="mmps", bufs=6, space="PSUM") as mmps, \
             tc.tile_pool(name="nups", bufs=1, space="PSUM") as nups, \
             tc.tile_pool(name="tps", bufs=1, space="PSUM") as tps:

            # ---- PE warmup: junk matmuls with no DMA dependency keep the
            # PE pstate ramping while wbig + sample 0 stream in.
            wu_sb = cst.tile([128, 512], BF16)
            nc.vector.memset(wu_sb[:], 1.0)
            wu_ps = tps.tile([128, 512], F32, tag="tp", name="wu_ps")
            for i in range(WARMUP_MM):
                nc.tensor.matmul(wu_ps[:], wu_sb[:, 0:128], wu_sb[:],
                                 start=(i == 0), stop=(i == WARMUP_MM - 1))
            wu_out = cst.tile([128, 8], F32)
            nc.scalar.copy(wu_out[:], wu_ps[:, 0:8])

            wbig_sb = wp.tile([128, CW + HC * H], BF16, name="wbig")
            xb_sb = xp.tile([128, SPC, HC, L], BF16, name="xb")
            ident = cst.tile([128, 128], F32)
            make_identity(nc, ident[:])

            # one big-line transfer for all weights/consts on the sync
            # queue. Transfers sharing a queue interleave (round-robin by
            # descriptor), so the startup-critical samples each get a
            # queue to themselves; the rest are issued in pairs from
            # later tail blocks (see the sample loop).
            nc.sync.dma_start(wbig_sb[:], wbig_d[:])
            nc.gpsimd.dma_start(xb_sb[:, 0], xb_d[:, 0])
            nc.scalar.dma_start(xb_sb[:, 1], xb_d[:, 1])
            nc.sync.dma_start(xb_sb[:, 2], xb_d[:, 2])

            # [128, u, i, m] fp8 view of the packed W_nu head: planes
            # are 16B apart, ldweights reads the first 8 of each
            wnu8 = None
            if N8C:
                wnu8 = wbig_sb[:, 2 * HC:CW].bitcast(F8).rearrange(
                    "p (u i m) -> p u i m", u=N8C // 2, i=2)[:, :, :, 0:NUM]

            def mm_tiles(s, ups8, upsb, trange):
                for t in trange:
                    ps = mmps.tile([128, L], F32, tag="mm")
                    for hc in range(HC):
                        nc.tensor.matmul(
                            ps[:],
                            wbig_sb[:, CW + t * H + hc * 128:
                                    CW + t * H + hc * 128 + 128],
                            xb_sb[:, s, hc, :],
                            start=(hc == 0), stop=(hc == HC - 1),
                        )
                    dst = ups8[:, t, :] if t < N8C else upsb[:, t - N8C, :]
                    nc.scalar.activation(
                        dst, ps[:], AF.Tanh, bias=wbig_sb[:, t:t + 1],
                    )

            def tail_nu(s, ups8, upsb):
                # nu*64: two fp8 DoubleRow passes over the low-|W_nu|
                # chunks + two bf16 passes over the high ones
                nu = nups.tile([NUM, L], F32, tag="nu")
                if N8C == 0:
                    nc.tensor.matmul(
                        nu[0:1, :], wbig_sb[:, HC:HC + 1], upsb[:, 0, :],
                        start=True, stop=False, skip_group_check=True,
                    )
                for u in range(N8C // 2):
                    nc.tensor.matmul(
                        nu[:], wnu8[:, u],
                        ups8[:, 2 * u:2 * u + 2, :],
                        start=(u == 0), stop=False,
                        perf_mode=DR, skip_group_check=True,
                    )
                for k in range(max(N8C, 1), HC):
                    nc.tensor.matmul(
                        nu[0:1, :], wbig_sb[:, HC + k:HC + k + 1],
                        upsb[:, k - N8C, :],
                        start=False, stop=(k == HC - 1),
                        skip_group_check=True,
                    )
                return nu

            def tail_soft(s, nu):
                # softmax over the 512 logits (single partition); exp's
                # scale folds away the x64 on W_nu. logits are small
                # enough that exp() needs no max subtraction.
                ex = smp.tile([1, L], BF16, tag="ex")
                z = smp.tile([1, 1], F32, tag="z")
                nc.scalar.activation(ex[:], nu[0:1, :], AF.Exp,
                                     scale=1.0 / 64.0, accum_out=z[:])
                rz = smp.tile([1, 1], F32, tag="rz")
                nc.vector.reciprocal(rz[:], z[:])

                # broadcast unnormalized E, pool in bf16, scale by 1/Z
                ab = smp.tile([128, L], BF16, tag="ab")
                nc.gpsimd.partition_broadcast(ab[:], ex[:])
                rzb = smp.tile([HC, 1], F32, tag="rzb")
                nc.gpsimd.partition_broadcast(rzb[:], rz[:], channels=HC)
                pooled_u = outp.tile([128, HC], F32, tag="pooled_u")
                for j in range(HC):
                    trash = smp.tile([128, L], BF16, tag="trash")
                    nc.vector.scalar_tensor_tensor(
                        trash[:],
                        xb_sb[:, s, j, :],
                        1.0,
                        ab[:],
                        ALU.mult,
                        ALU.mult,
                        accum_out=pooled_u[:, j:j + 1],
                    )

                return pooled_u, rzb

            def tail_drain(s, pooled_u, rzb):
                # pooled^T [128, HC] -> [HC, 128] -> DRAM row s
                tp = tps.tile([HC, 128], F32, tag="tp")
                nc.tensor.transpose(tp[:], pooled_u[:], ident[:])
                orow = outp.tile([HC, 128], F32, tag="orow")
                nc.scalar.activation(orow[:], tp[:], AF.Copy,
                                     scale=rzb[:, 0:1])
                nc.sync.dma_start(
                    out_d[s:s + 1, :].rearrange("o (c p) -> (o c) p", p=128),
                    orow[:],
                )

            deferred = {1: (3, 4), 2: (5, 6), 3: (7,)}
            prev = None      # (s, ups8, upsb)
            soft = {}        # s -> (pooled_u, rzb)
            for s in range(SPC):
                ups8 = None
                if N8C:
                    ups8 = upsp.tile([128, N8C, L], F8, tag="ups8", name="ups8")
                upsb = upsp.tile([128, HC - N8C, L], BF16, tag="upsb")
                mm_tiles(s, ups8, upsb, range(0, 2))
                if s - 2 in soft:
                    tail_drain(s - 2, *soft.pop(s - 2))
                if prev is not None:
                    pnu = tail_nu(prev[0], prev[1], prev[2])
                mm_tiles(s, ups8, upsb, range(2, HC))
                if prev is not None:
                    soft[prev[0]] = tail_soft(prev[0], pnu)
                for sd in deferred.get(s, ()):
                    nc.gpsimd.dma_start(xb_sb[:, sd], xb_d[:, sd])
                prev = (s, ups8, upsb)
            pnu = tail_nu(prev[0], prev[1], prev[2])
            tail_drain(SPC - 2, *soft.pop(SPC - 2))
            soft[SPC - 1] = tail_soft(SPC - 1, pnu)
            tail_drain(SPC - 1, *soft.pop(SPC - 1))

            nc.sync.dma_start(junk_d[:], wu_out[:])

    nc.finalize()
    return nc


def _prep_host(hidden_states, W_fc, b_fc, W_nu):
    bf = ml_dtypes.bfloat16
    f8 = ml_dtypes.float8_e4m3fn
    hs = np.ascontiguousarray(hidden_states, dtype=np.float32)
    W_fc = np.asarray(W_fc, np.float32)
    b_fc = np.asarray(b_fc, np.float32)
    W_nu = np.asarray(W_nu, np.float32)

    order = np.argsort(np.abs(W_nu), kind="stable")
    WT = W_fc.T[:, order]                                # [hin, kout sorted]
    wnu64 = (W_nu[order] * 64.0).reshape(HC, 128)        # [t, p]

    wbig = np.empty((128, CW + HC * H), dtype=bf)
    wbig[:, 0:HC] = b_fc[order].reshape(HC, 128).T.astype(bf)
    wbig[:, HC:2 * HC] = wnu64.T.astype(bf)
    # packed fp8 wnu for the DoubleRow nu: 16B [u, i] planes, wnu at m=0
    pk = wnu64[0:N8C].astype(f8).view(np.uint8)          # [4 chunks, 128]
    head = np.zeros((128, N8C, 16), np.uint8)
    head[:, :, 0] = pk.T
    wbig[:, 2 * HC:CW].view(np.uint8)[:] = head.reshape(128, N8C * 16)
    w = WT.reshape(HC, 128, HC, 128).transpose(1, 2, 0, 3)  # [p, t, hc, m]
    wbig[:, CW:] = np.ascontiguousarray(w).reshape(128, HC * H).astype(bf)

    xbs = []
    for c in range(NCORES):
        xt = hs[c * SPC:(c + 1) * SPC].reshape(SPC * L, H).T  # [H, TOK]
        v = xt.reshape(HC, 128, SPC, L).transpose(1, 2, 0, 3)  # [p,s,j,l]
        xbs.append(np.ascontiguousarray(v).astype(bf))
    return wbig, xbs


def kernel(hidden_states, W_fc, b_fc, W_nu, _trace=False, _trace_kwargs=None):
    from concourse.bass_utils import run_bass_kernel_spmd

    wbig, xbs = _prep_host(hidden_states, W_fc, b_fc, W_nu)
    in_maps = [{"xb": xbs[c], "wbig": wbig} for c in range(NCORES)]

    if "nc" not in _compiled:
        _compiled["nc"] = _build()
    res = run_bass_kernel_spmd(
        _compiled["nc"], in_maps, list(range(NCORES)),
        trace=_trace, **(_trace_kwargs or {}),
    )
    kernel.last_results = res
    out = np.concatenate([np.asarray(r["out"], np.float32) for r in res.results])
    return out
